# revision 19
# baseline (speedup 1.0000x reference)
"""GATv2 layer (nn_GATv2Layer_12979391169461) Trainium2 Bass kernel — v7.1.

Reference math (N=2048, F=128, HEADS=8, OUT_DIM=8, alpha=0.2):
    h  = (X @ W).reshape(N, 8, 8)
    s1 = h . a1 ; s2 = h . a2                      # [N, 8]
    e[n,j,k]   = lrelu(s1[n,k] + s2[j,k]) masked by A[n,j] (-1e9)
    att[n,j,k] = softmax_j(e[n,j,k])
    out[n,j,d] = sum_k att[n,j,k] * h[n,k,d]       # contracts the HEAD axis
    return lrelu(out).reshape(N*N/8, 64)

v7 design (per 16-row block, p = n_local*8 + head partition layout):
  * Softmax-scale invariance per (n,k) cancels exp(s1):
      num ~ max(exp(-0.8*s1) * exp(0.2*s2), exp(s2)),   exp(s2) = exp(0.2*s2)^5
    so numerator+mask+denominator for a block is ONE custom DVE op
    (registered into the ant custom-DVE table at import):
      q = max(C0*Src0, Src0^5) * Src1,  accum_out = sum_j q
    with Src0 = e02 table (exp(0.2*s2), PE-replicated layout, computed once),
    Src1 = the 0/1 mask, C0 = exp(-0.8*s1) per-partition column.
  * The mask arrives pre-replicated across heads straight from HBM via a
    broadcast-read DMA descriptor (reads each row 8x) -> SBUF.
    No PE replication matmul, no PSUM mask residency.
  * Aggregation: block-diagonal [128,128] fp16 x [128,2048] fp16 matmul with
    1/denominator folded into the stationary (one small divide-stt per block).
  * Final lrelu: one ACT Prelu per block (PSUM fp32 -> SBUF fp16); output is
    written fp16 (halves HBM write traffic; host upcasts).
Each of the 8 cores owns 256 rows. Device rows are (block, n_local, d) x (j);
the host transposes while unsharding.
"""

import os
import sys
from contextlib import ExitStack
from operator import add as _op_add

import numpy as np

sys.path.insert(0, "/opt/trn_rl_repo")

import concourse.tile as tile  # noqa: E402
from concourse import bacc, mybir  # noqa: E402
from concourse import dve_ops as _dve_ops  # noqa: E402
from concourse.dve_spec import Spec, Src0, Src1, C0, maxx, sq, lower  # noqa: E402
from concourse.dve_spec import _has_src1  # noqa: E402
from concourse.dve_uop import DveOpSpec  # noqa: E402
from concourse.bass_utils import run_bass_kernel_spmd  # noqa: E402

N, F = 2048, 128
HEADS, OUT_DIM = 8, 8
ALPHA = 0.2
NCORES = 8
ROWS = N // NCORES          # 256 own rows per core
BLOCKS = ROWS // 16         # 16 blocks of 16 rows
FP = mybir.dt.float32
F16 = mybir.dt.float16
F8 = mybir.dt.float8e4
AOP = mybir.AluOpType

MASK8 = os.environ.get("GAT_MASK8", "1") == "1"   # fp8 masks for the custom op
USE_DIV = os.environ.get("GAT_DIV", "0") == "1"   # divide in wblk stt (no ISA support)


def _register_gat_op():
    """q = max(C0*Src0, Src0^5) * Src1 ; accum_out = sum(q)."""
    name = "GAT_SCORE_MASK_ANT"
    for op in _dve_ops.OPS:
        if op.name == name:
            return op
    spec = Spec(body=maxx(Src0 * C0, sq(sq(Src0)) * Src0) * Src1, accum=_op_add)
    row = _dve_ops._CUSTOM_DVE_ROW_BASE + len(_dve_ops.OPS)
    shas = {}
    for ver in ("v3", "v4"):
        s = DveOpSpec(name=name, opcode=row, uops=lower(spec, ver=ver),
                      rd1_en=_has_src1(spec))
        shas[ver] = s.sha(ver)
    op = _dve_ops.DveOp(name, spec, subdim=False, uops_sha=shas)
    _dve_ops.OPS.append(op)
    _dve_ops._SUB_OPCODE_FOR_NAME[name] = row
    _dve_ops.CUSTOM_DVE_SPECS[name] = spec
    return op


GAT_OP = _register_gat_op()


def build_program():
    nc = bacc.Bacc("TRN2", debug=False)

    xt_d = nc.dram_tensor("XT", [F, N], F16, kind="ExternalInput")
    xto_d = nc.dram_tensor("XTo", [F, ROWS], F16, kind="ExternalInput")
    w_d = nc.dram_tensor("Wmat", [F, 64], F16, kind="ExternalInput")
    wa1_d = nc.dram_tensor("WA1", [F, HEADS], F16, kind="ExternalInput")
    wa2r_d = nc.dram_tensor("WA2R", [F, 128], F16, kind="ExternalInput")
    mask_d = nc.dram_tensor("MASKR", [ROWS, N], F8 if MASK8 else F16,
                            kind="ExternalInput")
    bd_d = nc.dram_tensor("BD_MASK", [128, 128], F16, kind="ExternalInput")
    id_d = nc.dram_tensor("IDENT", [128, 128], FP, kind="ExternalInput")
    out_d = nc.dram_tensor("OUTC", [ROWS * 8, N], F16, kind="ExternalOutput")

    with ExitStack() as ctx:
        tc = ctx.enter_context(tile.TileContext(nc))
        per = ctx.enter_context(tc.tile_pool(name="persist", bufs=1))
        e02_rep = per.tile([128, N], F16, tag="e02")
        # w1_nmaj holds exp(-0.8*s1) values (WA1 is host-scaled by -0.8)
        w1_nmaj = [per.tile([128, HEADS], FP, tag=f"wn{i}", name=f"wn{i}")
                   for i in range(2)]
        h_nmaj = [per.tile([128, 64], F16, tag=f"hn{i}", name=f"hn{i}")
                  for i in range(2)]
        bd_mask = per.tile([128, 128], F16, tag="bd")
        alpha_v = per.tile([128, 1], FP, tag="al")
        nc.vector.memset(alpha_v[:], ALPHA)

        sb_m = ctx.enter_context(tc.tile_pool(name="blkm", bufs=4))
        sb_q = ctx.enter_context(tc.tile_pool(name="blkq", bufs=4))
        sb_o = ctx.enter_context(tc.tile_pool(name="blko", bufs=4))
        sb_w = ctx.enter_context(tc.tile_pool(name="blkw", bufs=4))
        sb_s = ctx.enter_context(tc.tile_pool(name="blks", bufs=8))

        # ---------------- preprocessing ----------------
        with tc.tile_pool(name="pre", bufs=1) as pre, \
             tc.tile_pool(name="pre_ps", bufs=1, space="PSUM") as pre_ps:
            # own-row stats first: they head the w1/h critical path
            xto = pre.tile([F, ROWS], F16)
            nc.gpsimd.dma_start(xto[:], xto_d.ap())
            wa1 = pre.tile([F, HEADS], F16)
            nc.gpsimd.dma_start(wa1[:], wa1_d.ap())
            ident = pre.tile([128, 128], FP)
            nc.gpsimd.dma_start(ident[:], id_d.ap())
            wmat = pre.tile([F, 64], F16)
            nc.gpsimd.dma_start(wmat[:], w_d.ap())
            wa2r = pre.tile([F, 128], F16)
            nc.gpsimd.dma_start(wa2r[:], wa2r_d.ap())
            nc.gpsimd.dma_start(bd_mask[:], bd_d.ap())
            xt = pre.tile([F, N], F16)
            for c in range(8):
                nc.sync.dma_start(xt[:, c * 256:(c + 1) * 256],
                                  xt_d.ap()[:, c * 256:(c + 1) * 256])

            s1o_ps = pre_ps.tile([HEADS, ROWS], FP, tag="small2")
            nc.tensor.matmul(s1o_ps[:], wa1[:], xto[:], start=True, stop=True)
            s1o = pre.tile([HEADS, ROWS], FP)
            nc.scalar.copy(s1o[:], s1o_ps[:])
            hto_ps = pre_ps.tile([64, ROWS], FP, tag="small")
            nc.tensor.matmul(hto_ps[:], wmat[:], xto[:], start=True, stop=True)
            hto = pre.tile([64, ROWS], FP)
            nc.scalar.copy(hto[:], hto_ps[:])

            # e02 table computed directly in x16-replicated layout [128, N];
            # exp per 512-col chunk to pipeline behind the s2t matmuls
            s2t_ps = pre_ps.tile([128, N], FP, tag="big")
            for c in range(8):
                sl = slice(c * 256, (c + 1) * 256)
                nc.tensor.matmul(s2t_ps[:, sl], wa2r[:], xt[:, sl],
                                 start=True, stop=True)
                nc.scalar.activation(e02_rep[:, sl], s2t_ps[:, sl],
                                     mybir.ActivationFunctionType.Exp, scale=ALPHA)

            # n-major transposed stats: w1_nmaj = exp(-0.8*s1), h_nmaj = h
            for i in range(2):
                tp = pre_ps.tile([128, HEADS], FP, tag="tiny")
                nc.tensor.transpose(tp[:], s1o[:, i * 128:(i + 1) * 128],
                                    ident[:HEADS, :HEADS])
                nc.scalar.activation(w1_nmaj[i][:], tp[:],
                                     mybir.ActivationFunctionType.Exp)
                tp2 = pre_ps.tile([128, 64], FP, tag="tiny")
                nc.tensor.transpose(tp2[:], hto[:, i * 128:(i + 1) * 128],
                                    ident[:64, :64])
                nc.scalar.copy(h_nmaj[i][:], tp2[:])

        # ---------------- main loop over 16-row blocks ----------------
        ps_y = ctx.enter_context(tc.tile_pool(name="psy", bufs=2, space="PSUM"))
        for b in range(BLOCKS):
            half, row = divmod(b * 16, 128)
            # mask rows, replicated x8 across heads by the DMA read pattern
            mrep = sb_m.tile([128, N], F8 if MASK8 else F16, tag="m")
            msrc = mask_d.ap()[b * 16:(b + 1) * 16]
            nc.sync.dma_start(mrep[:], msrc[:, None, :].broadcast_to([16, 8, N]))
            # per-block staging: [16,8]->[128,1] and [16,64]->[128,8]
            w1c = sb_s.tile([128, 1], FP, tag="w1c")
            nc.gpsimd.dma_start(w1c[:], w1_nmaj[half][row:row + 16, :])
            hb_t = sb_s.tile([128, HEADS], F16, tag="hb")
            nc.gpsimd.dma_start(hb_t[:], h_nmaj[half][row:row + 16, :])

            # q = max(w1*e02, e02^5) * mask ; dq = sum_j q  — ONE fused DVE op.
            # Blocks 0-1 run in column halves so the DVE starts on the first
            # half of the e02 table while the second half is still being built.
            q = sb_q.tile([128, N], F16, tag="q")
            dq = sb_s.tile([128, 1], FP, tag="dq")
            if b < 4 or b >= 14:
                dqh = sb_s.tile([128, 2], FP, tag="dqh")
                for hh in range(2):
                    sl = slice(hh * 1024, (hh + 1) * 1024)
                    nc.vector._custom_dve(GAT_OP, out=q[:, sl],
                                          in0=e02_rep[:, sl], in1=mrep[:, sl],
                                          s0=w1c[:], accum_out=dqh[:, hh:hh + 1])
                nc.vector.tensor_tensor(dq[:], dqh[:, 0:1], dqh[:, 1:2],
                                        op=AOP.add)
            else:
                nc.vector._custom_dve(GAT_OP, out=q[:], in0=e02_rep[:],
                                      in1=mrep[:], s0=w1c[:], accum_out=dq[:])

            # W_blk[p=nh, f=n'd] = h_own[n,h*8+d]/dq[nh] * blockdiag(n==n')
            wblk = sb_w.tile([128, 128], F16, tag="wblk")
            if USE_DIV:
                nc.vector.scalar_tensor_tensor(
                    wblk[:].rearrange("p (o e) -> p o e", o=16),
                    hb_t[:].rearrange("p (o e) -> p o e", o=1)
                        .broadcast_to([128, 16, HEADS]),
                    dq[:],
                    bd_mask[:].rearrange("p (o e) -> p o e", o=16),
                    op0=AOP.divide, op1=AOP.mult)
            else:
                rdq = sb_s.tile([128, 1], FP, tag="rdq")
                nc.vector.reciprocal(rdq[:], dq[:])
                nc.vector.scalar_tensor_tensor(
                    wblk[:].rearrange("p (o e) -> p o e", o=16),
                    hb_t[:].rearrange("p (o e) -> p o e", o=1)
                        .broadcast_to([128, 16, HEADS]),
                    rdq[:],
                    bd_mask[:].rearrange("p (o e) -> p o e", o=16),
                    op0=AOP.mult, op1=AOP.mult)

            # y[p=nd, j] = sum_h W_blk[nh, nd] q[nh, j] ; out = lrelu(y) fp16
            y_ps = ps_y.tile([128, N], FP, tag="y")
            for c in range(4):
                nc.tensor.matmul(y_ps[:, c * 512:(c + 1) * 512], wblk[:],
                                 q[:, c * 512:(c + 1) * 512], start=True, stop=True)
            out_sb = sb_o.tile([128, N], F16, tag="out")
            if b >= 14:
                for hh in range(2):
                    sl = slice(hh * 1024, (hh + 1) * 1024)
                    nc.scalar.activation(out_sb[:, sl], y_ps[:, sl],
                                         mybir.ActivationFunctionType.Prelu,
                                         alpha=alpha_v[:])
                    nc.sync.dma_start(out_d.ap()[b * 128:(b + 1) * 128, sl],
                                      out_sb[:, sl])
            else:
                nc.scalar.activation(out_sb[:], y_ps[:],
                                     mybir.ActivationFunctionType.Prelu,
                                     alpha=alpha_v[:])
                nc.sync.dma_start(out_d.ap()[b * 128:(b + 1) * 128, :], out_sb[:])

    nc.compile()
    return nc


_NC_CACHE = None


def _get_program():
    global _NC_CACHE
    if _NC_CACHE is None:
        _NC_CACHE = build_program()
    return _NC_CACHE


def _host_inputs(X, A, W, attn_kernel):
    import ml_dtypes

    XT = np.ascontiguousarray(X.T).astype(np.float32)
    a1 = attn_kernel[:OUT_DIM, 0].astype(np.float32)
    a2 = attn_kernel[OUT_DIM:, 0].astype(np.float32)
    Wf = W.astype(np.float32).reshape(F, HEADS, OUT_DIM)
    WA1 = np.ascontiguousarray(Wf @ a1) * -0.8           # [F, HEADS], -0.8 folded
    WA2 = Wf @ a2                                        # [F, HEADS]
    WA2R = np.ascontiguousarray(np.tile(WA2, (1, 16)))   # [F, 128]
    BD = np.zeros((128, 128), np.float32)
    for nl in range(16):
        BD[nl * 8:(nl + 1) * 8, nl * 8:(nl + 1) * 8] = 1.0
    IDENT = np.eye(128, dtype=np.float32)

    mdt = ml_dtypes.float8_e4m3 if MASK8 else np.float16
    Af = (A > 0).astype(np.float32)
    in_maps = []
    for c in range(NCORES):
        n0 = c * ROWS
        in_maps.append({
            "XT": XT.astype(np.float16),
            "XTo": np.ascontiguousarray(XT[:, n0:n0 + ROWS]).astype(np.float16),
            "Wmat": W.astype(np.float16),
            "WA1": WA1.astype(np.float16),
            "WA2R": WA2R.astype(np.float16),
            "MASKR": Af[n0:n0 + ROWS].astype(mdt),
            "BD_MASK": BD.astype(np.float16),
            "IDENT": IDENT,
        })
    return in_maps


def kernel(X, A, W, attn_kernel, _want_timing=False):
    X = np.asarray(X)
    A = np.asarray(A)
    W = np.asarray(W)
    attn_kernel = np.asarray(attn_kernel)
    nc = _get_program()
    in_maps = _host_inputs(X, A, W, attn_kernel)
    res = None
    last_err = None
    for attempt in range(3):
        try:
            res = run_bass_kernel_spmd(nc, in_maps, core_ids=list(range(NCORES)),
                                       trace=_want_timing)
            break
        except Exception as e:  # transient NRT device-unrecoverable: retry
            last_err = e
            import time
            time.sleep(2.0)
    if res is None:
        raise last_err
    # device rows are (block, n_local, d) x (j); reference wants (n, j, d)
    parts = []
    for c in range(NCORES):
        oc = np.asarray(res.results[c]["OUTC"], dtype=np.float32)
        oc = oc.reshape(BLOCKS, 16, OUT_DIM, N)            # [b, nl, d, j]
        oc = oc.transpose(0, 1, 3, 2).reshape(-1, OUT_DIM * HEADS)
        parts.append(oc)
    out = np.concatenate(parts, axis=0)
    if _want_timing:
        return out, res
    return out
